# revision 28
# baseline (speedup 1.0000x reference)
"""Trainium2 Bass kernel for GQA attention block (B=2, S=2048, HS=2048, H=16, HKV=4, D=128).

Strategy (8 NeuronCores, SPMD):
  - Head-parallel: core c computes q-heads {2c, 2c+1} and kv-head c//2 for BOTH batches.
  - Fused QKV projection: one 512-wide rhs stream [q0|q1|k|v] per contraction tile.
  - Per-head RMS norm + RoPE in [tok, d] layout; rope reads PSUM directly on VectorE;
    the norm multiply is folded into the PE transpose by using diag(1/rms) as the
    transpose's streaming operand (norm weights and 1/sqrt(D) folded into
    host-precomputed cos/sin tables).
  - Causal flash attention in transposed layout: S^T = K_rope @ Q_rope^T ([kv, q]),
    exp on ScalarE (no max subtraction needed: |scores| <= sqrt(D)), diagonal blocks
    narrowed to the causal triangle (matmul/exp/mask only over valid columns),
    O^T = V^T @ P^T accumulated in PSUM. Softmax denominators: GpSimd C-axis
    tensor_reduce for full blocks (gathered in a [16,512] tile, folded in with one
    small matmul), narrowed ones-matmuls for diagonal blocks.
  - One 8-rank AllToAll per head redistributes head-shards -> (batch, seq-strip)
    shards; output projection per strip; host concatenates the 8 strips.
"""

import sys

sys.path.insert(0, "/opt/trn_rl_repo")

import numpy as np
import ml_dtypes

BF16 = ml_dtypes.bfloat16

B, H, HKV, D = 2, 16, 4, 128
EPS = 1e-6
P = 128
N_CORES = 8


def build(S=2048, HS=2048):
    """Build + compile the SPMD graph. Returns the Bacc module."""
    import concourse.bacc as bacc
    import concourse.tile as tile
    import concourse.mybir as mybir

    dt = mybir.dt
    f32 = dt.float32
    bf16 = dt.bfloat16
    AF = mybir.ActivationFunctionType
    ALU = mybir.AluOpType
    AX = mybir.AxisListType

    T = S // P          # tok tiles per batch
    M = 2 * T           # tok tiles total (2 batches)
    KT = HS // P        # contraction tiles for qkv projection
    KO = (H * D) // P   # contraction tiles for o projection (16)
    CW = S // 4         # q-chunk width == strip width
    CB = CW // P        # kv blocks per chunk step
    OCH = HS // 512     # output column chunks
    NQ = 2              # q heads per core

    nc = bacc.Bacc("TRN2", target_bir_lowering=False, debug=False,
                   enable_asserts=True, num_devices=N_CORES)

    xT = nc.dram_tensor("xT", [M, P, HS], bf16, kind="ExternalInput")
    wqkvT = nc.dram_tensor("wqkvT", [P, KT * 384], bf16, kind="ExternalInput")
    woT = nc.dram_tensor("woT", [P, KO * HS], bf16, kind="ExternalInput")
    cosq_d = nc.dram_tensor("cosq", [P, T * D], bf16, kind="ExternalInput")
    sinq_d = nc.dram_tensor("sinq", [P, T * D], bf16, kind="ExternalInput")
    cosk_d = nc.dram_tensor("cosk", [P, T * D], bf16, kind="ExternalInput")
    sink_d = nc.dram_tensor("sink", [P, T * D], bf16, kind="ExternalInput")
    tri_d = nc.dram_tensor("tri", [P, P], bf16, kind="ExternalInput")
    onesq_d = nc.dram_tensor("onesq", [P, P], bf16, kind="ExternalInput")
    ident_d = nc.dram_tensor("ident", [P, P], bf16, kind="ExternalInput")
    out_d = nc.dram_tensor("out", [CW, HS], bf16, kind="ExternalOutput")

    with tile.TileContext(nc) as tc:
        with tc.tile_pool(name="const", bufs=1) as cpool, \
             tc.tile_pool(name="weights", bufs=1) as wpool, \
             tc.tile_pool(name="qkv", bufs=1) as qkvpool, \
             tc.tile_pool(name="dram", bufs=1, space="DRAM") as dpool:

            # startup-critical DMA order: wqkv in per-k pieces so the first
            # projection matmuls can start as soon as k=0 lands; rope tables and
            # ident before the first tile's norm/rope; attention-only constants
            # (tri/onesq) last.
            # KV-split: core pairs split the kv-head work — even cores project
            # [q0|q1|k], odd cores [q0|q1|v] (384 wide instead of 512); the
            # processed k^T and raw v are then exchanged pairwise via a 2-rank
            # AllGather, hidden under surrounding compute.
            wqkv_sb = wpool.tile([P, KT, 384], bf16, name="wqkv_sb")
            wqkv_src = wqkvT.ap().rearrange("p (k f) -> p k f", k=KT)

            cosq_sb = cpool.tile([P, T, D], bf16, name="cosq_sb")
            sinq_sb = cpool.tile([P, T, D], bf16, name="sinq_sb")
            cosk_sb = cpool.tile([P, T, D], bf16, name="cosk_sb")
            sink_sb = cpool.tile([P, T, D], bf16, name="sink_sb")
            tri_sb = cpool.tile([P, P], bf16, name="tri_sb")
            onesq_sb = cpool.tile([P, P], bf16, name="onesq_sb")
            ident_sb = cpool.tile([P, P], bf16, name="ident_sb")
            eps_sb = cpool.tile([P, 1], f32, name="eps_sb")
            nc.gpsimd.memset(eps_sb[:], EPS)
            scr_sb = cpool.tile([P, 1], f32, name="scr_sb")
            # prewarm the ACT Exp table so its load isn't on the critical path
            # at the stage-A -> attention boundary
            nc.scalar.activation(scr_sb, eps_sb, AF.Exp)

            # per-batch tiles so batch-0 attention can overlap batch-1 stage A.
            # kT_b/v_b hold this core's computed X (k-path / v-path of its X);
            # kTx_b/vx_b hold the pair-exchanged real k^T and v.
            qT_b = [qkvpool.tile([P, NQ, S], bf16, name=f"qT_b{i}")
                    for i in range(2)]
            kT_b = [qkvpool.tile([P, S], bf16, name=f"kT_b{i}")
                    for i in range(2)]
            v_b = [qkvpool.tile([P, T, D], bf16, name=f"v_b{i}")
                   for i in range(2)]
            kTx_b = [qkvpool.tile([P, S], bf16, name=f"kTx_b{i}")
                     for i in range(2)]
            vx_b = [qkvpool.tile([P, T, D], bf16, name=f"vx_b{i}")
                    for i in range(2)]

            a2a_in = [dpool.tile([1024, CW], bf16, name=f"a2a_in{h}")
                      for h in range(NQ)]
            a2a_out = [dpool.tile([1024, CW], bf16, name=f"a2a_out{h}")
                       for h in range(NQ)]
            exi = [dpool.tile([P, 2 * S], bf16, name=f"exi{i}")
                   for i in range(2)]
            exo = [dpool.tile([2 * P, 2 * S], bf16, name=f"exo{i}")
                   for i in range(2)]

            # ---------------- stage 1+2: QKV projection, RMS norm, RoPE, transpose
            with tc.tile_pool(name="s12", bufs=2) as s12, \
                 tc.tile_pool(name="xin", bufs=6) as xin, \
                 tc.tile_pool(name="ps12", bufs=2, space="PSUM") as ps12:
                xms = {}

                def load_xm(m):
                    t_ = xin.tile([P, KT, P], bf16, tag="xm", name=f"xm{m}")
                    nc.sync.dma_start(t_[:],
                                      xT.ap()[m].rearrange("p (k t) -> p k t", k=KT))
                    xms[m] = t_

                load_xm(0)
                for k in range(KT):
                    nc.sync.dma_start(wqkv_sb[:, k, :], wqkv_src[:, k, :])
                nc.sync.dma_start(cosq_sb[:],
                                  cosq_d.ap().rearrange("p (t d) -> p t d", t=T))
                nc.sync.dma_start(sinq_sb[:],
                                  sinq_d.ap().rearrange("p (t d) -> p t d", t=T))
                nc.sync.dma_start(cosk_sb[:],
                                  cosk_d.ap().rearrange("p (t d) -> p t d", t=T))
                nc.sync.dma_start(sink_sb[:],
                                  sink_d.ap().rearrange("p (t d) -> p t d", t=T))
                nc.sync.dma_start(ident_sb[:], ident_d.ap())
                for m in range(1, 5):
                    load_xm(m)
                nc.sync.dma_start(tri_sb[:], tri_d.ap())
                nc.sync.dma_start(onesq_sb[:], onesq_d.ap())

                pend = None  # previous tile's (ro, rok, diagr, b, col): transposes
                # are deferred one iteration so the norm/rope chain of tile m
                # overlaps the projection matmuls of tile m+1 (keeps PE dense).

                def emit_transposes(p):
                    ro_, rok_, diagr_, b_, col_ = p
                    for idx in range(3):
                        srct = [ro_[:, 0:P], ro_[:, P:2 * P], rok_][idx]
                        dst = qT_b[b_][:, idx, col_:col_ + P] if idx < 2 \
                            else kT_b[b_][:, col_:col_ + P]
                        tp = ps12.tile([P, P], f32, tag="tp")
                        nc.tensor.matmul(tp, srct, diagr_[:, idx, :],
                                         start=True, stop=True)
                        nc.scalar.copy(dst, tp)

                def emit_exchange(b_):
                    nc.sync.dma_start(exi[b_][:, 0:S], kT_b[b_][:])
                    nc.sync.dma_start(exi[b_][:, S:2 * S],
                                      v_b[b_][:].rearrange("p t d -> p (t d)"))
                    nc.gpsimd.collective_compute(
                        "AllGather", mybir.AluOpType.bypass,
                        ins=[exi[b_][:].opt()], outs=[exo[b_][:].opt()],
                        replica_groups=[[2 * i, 2 * i + 1]
                                        for i in range(N_CORES // 2)],
                    )
                    nc.sync.dma_start(kTx_b[b_][:], exo[b_][0:P, 0:S])
                    nc.sync.dma_start(
                        vx_b[b_][:],
                        exo[b_][P:2 * P, S:2 * S].rearrange("p (t d) -> p t d",
                                                            t=T))

                for m in range(M):
                    b, mm = m // T, m % T
                    if m + 5 < M:
                        load_xm(m + 5)
                    xm = xms.pop(m)
                    ps_qkv = ps12.tile([P, 384], f32, tag="qkv")
                    for k in range(KT):
                        nc.tensor.matmul(ps_qkv, xm[:, k, :], wqkv_sb[:, k, :],
                                         start=(k == 0), stop=(k == KT - 1))
                    # X as V: plain copy to [tok, d] layout (VectorE; reads PSUM)
                    nc.vector.tensor_copy(v_b[b][:, mm, :], ps_qkv[:, 256:384])
                    col = P * mm
                    # squared-sums for q0|q1|k read straight from PSUM (ScalarE)
                    sq = s12.tile([P, P], bf16, tag="sq")
                    ssum = s12.tile([P, 3], f32, tag="ssum")
                    for idx in range(3):
                        nc.scalar.activation(sq, ps_qkv[:, idx * P:(idx + 1) * P],
                                             AF.Square,
                                             accum_out=ssum[:, idx:idx + 1])
                    rms = s12.tile([P, 3], f32, tag="rms")
                    nc.scalar.activation(rms, ssum, AF.Sqrt,
                                         bias=eps_sb[:], scale=1.0 / D)
                    rinv = s12.tile([P, 3], f32, tag="rinv")
                    nc.vector.reciprocal_approx_fast(rinv, rms)
                    # diag(1/rms) per head: folded into the transpose matmuls
                    diagr = s12.tile([P, 3, P], bf16, tag="diagr")
                    for idx in range(3):
                        nc.vector.tensor_scalar_mul(diagr[:, idx, :], ident_sb,
                                                    rinv[:, idx:idx + 1])
                    # RoPE on raw q/k straight from PSUM; q heads batched via
                    # duplicated tables. (rope commutes with the per-token norm
                    # scalar, which is applied by the diag-transpose below.)
                    ro = s12.tile([P, 2 * P], bf16, tag="ro")
                    ro_v = ro.rearrange("p (h d) -> p h d", h=2)
                    ps_q_h = ps_qkv[:, 0:2 * P].rearrange("p (h d) -> p h d", h=2)
                    cos_b = cosq_sb[:, mm, None, :].to_broadcast((P, 2, D))
                    nc.vector.tensor_tensor(ro_v, ps_q_h, cos_b, ALU.mult)
                    rh = s12.tile([P, 2 * P], bf16, tag="rh")
                    rh_v = rh.rearrange("p (h x d) -> p h x d", h=2, x=2)
                    ps_q_x = ps_qkv[:, 0:2 * P].rearrange("p (h x d) -> p h x d",
                                                          h=2, x=2)
                    sinq_mm = sinq_sb[:, mm, :].rearrange("p (x d) -> p x d", x=2)
                    sinA = sinq_mm[:, None, 0, :].to_broadcast((P, 2, 64))
                    sinB = sinq_mm[:, None, 1, :].to_broadcast((P, 2, 64))
                    nc.vector.tensor_tensor(rh_v[:, :, 0, :], ps_q_x[:, :, 1, :],
                                            sinA, ALU.mult)
                    nc.vector.tensor_tensor(rh_v[:, :, 1, :], ps_q_x[:, :, 0, :],
                                            sinB, ALU.mult)
                    nc.vector.tensor_tensor(ro, ro, rh, ALU.add)
                    # RoPE for k
                    rok = s12.tile([P, P], bf16, tag="rok")
                    nc.vector.tensor_tensor(rok, ps_qkv[:, 2 * P:3 * P],
                                            cosk_sb[:, mm, :], ALU.mult)
                    rhk = s12.tile([P, P], bf16, tag="rhk")
                    nc.vector.tensor_tensor(rhk[:, 0:64], ps_qkv[:, 2 * P + 64:3 * P],
                                            sink_sb[:, mm, 0:64], ALU.mult)
                    nc.vector.tensor_tensor(rhk[:, 64:128], ps_qkv[:, 2 * P:2 * P + 64],
                                            sink_sb[:, mm, 64:128], ALU.mult)
                    nc.vector.tensor_tensor(rok, rok, rhk, ALU.add)
                    if pend is not None:
                        emit_transposes(pend)
                    if m == T:
                        emit_exchange(0)  # batch-0 k/v pair exchange, hidden
                        # under batch-1 stage-A compute
                    pend = (ro, rok, diagr, b, col)
                emit_transposes(pend)
                emit_exchange(1)

            # prefetch o-projection weights during attention
            wo_sb, _wo_free = tc.tile([P, KO, HS], bf16, name="wo_sb")
            wo_src = woT.ap().rearrange("p (k f) -> p k f", k=KO)
            for k4 in range(0, KO, 4):
                nc.sync.dma_start(wo_sb[:, k4:k4 + 4, :], wo_src[:, k4:k4 + 4, :])

            # ---------------- stage 3: causal attention, head-major so each
            # head's AllToAll overlaps the next head's compute
            s4_ctx = tc.tile_pool(name="s4", bufs=1)
            s4 = s4_ctx.__enter__()
            with tc.tile_pool(name="s3", bufs=6) as s3, \
                 tc.tile_pool(name="s3b", bufs=2) as s3b, \
                 tc.tile_pool(name="ps3", bufs=1, space="PSUM") as ps3:
                attn_sb = []
                for h in range(NQ):
                    for bb in range(2):
                        for c in (3, 2, 1, 0):
                            qv = qT_b[bb][:, h, CW * c: CW * (c + 1)]
                            nb = (c + 1) * CB
                            nd = nb - CB  # non-diagonal blocks (full width)
                            o_ps = ps3.tile([P, CW], f32, tag="o", bufs=2)
                            sum_ps = ps3.tile([P, CW], f32, tag="sum", bufs=2)

                            def emit_acc(kb, pT, off, vb=vx_b[bb], nb=nb):
                                # denominator + O accumulation for block kb;
                                # deferred one block so the next score matmul
                                # hides the exp/mask latency (keeps PE dense).
                                nc.tensor.matmul(sum_ps[:, off:], onesq_sb,
                                                 pT[:, off:], start=(kb == 0),
                                                 stop=(kb == nb - 1))
                                nc.tensor.matmul(o_ps[:, off:],
                                                 vb[:, kb, :], pT[:, off:],
                                                 start=(kb == 0),
                                                 stop=(kb == nb - 1))

                            pend = None
                            for kb in range(nb):
                                j = kb - nd  # diagonal index, >= 0 for diag blocks
                                off = P * j if j >= 0 else 0
                                s_ps = ps3.tile([P, CW], f32, tag="s", bufs=3)
                                nc.tensor.matmul(
                                    s_ps[:, off:],
                                    kTx_b[bb][:, P * kb: P * (kb + 1)],
                                    qv[:, off:], start=True, stop=True)
                                pT = s3.tile([P, CW], bf16, tag="pT")
                                nc.scalar.activation(pT[:, off:], s_ps[:, off:],
                                                     AF.Exp)
                                if j >= 0:
                                    # causal triangle on the first P columns of
                                    # this diagonal block
                                    nc.vector.tensor_tensor(
                                        pT[:, off:off + P], pT[:, off:off + P],
                                        tri_sb, ALU.mult)
                                if pend is not None:
                                    emit_acc(*pend)
                                pend = (kb, pT, off)
                            emit_acc(*pend)
                            rec = s3b.tile([P, CW], f32, tag="rec")
                            nc.vector.reciprocal_approx_fast(rec, sum_ps)
                            o_sb = s3b.tile([P, CW], bf16, tag="o_sb")
                            nc.vector.tensor_tensor(o_sb, o_ps, rec, ALU.mult)
                            r0 = P * (4 * bb + c)
                            nc.sync.dma_start(a2a_in[h][r0:r0 + P, :], o_sb)
                    nc.gpsimd.collective_compute(
                        "AllToAll", mybir.AluOpType.bypass,
                        ins=[a2a_in[h][:].opt()],
                        outs=[a2a_out[h][:].opt()],
                        replica_groups=[list(range(N_CORES))],
                    )
                    a_sb = s4.tile([P, KO // NQ, CW], bf16, name=f"attn_sb{h}")
                    a2a_view = a2a_out[h][:].rearrange("(k p) t -> p k t", p=P)
                    for k8 in range(KO // NQ):
                        nc.sync.dma_start(a_sb[:, k8, :], a2a_view[:, k8, :])
                    attn_sb.append(a_sb)

            # ---------------- stage 4: output projection for this core's strip
            with tc.tile_pool(name="s4o", bufs=2) as s4o, \
                 tc.tile_pool(name="ps4", bufs=2, space="PSUM") as ps4:
                accs = {}
                for t in range(CW // P):
                    for oc in range(OCH):
                        ps_o = ps4.tile([P, 512], f32, tag="oproj")
                        for k8 in range(KO // NQ):
                            nc.tensor.matmul(
                                ps_o, attn_sb[0][:, k8, P * t:P * (t + 1)],
                                wo_sb[:, NQ * k8, 512 * oc:512 * (oc + 1)],
                                start=(k8 == 0), stop=(k8 == KO // NQ - 1))
                        acc = s4o.tile([P, 512], bf16, tag="acc", bufs=16)
                        nc.scalar.copy(acc, ps_o)
                        accs[(t, oc)] = acc
                for t in range(CW // P):
                    for oc in range(OCH):
                        ps_o = ps4.tile([P, 512], f32, tag="oproj")
                        for k8 in range(KO // NQ):
                            nc.tensor.matmul(
                                ps_o, attn_sb[1][:, k8, P * t:P * (t + 1)],
                                wo_sb[:, NQ * k8 + 1, 512 * oc:512 * (oc + 1)],
                                start=(k8 == 0), stop=(k8 == KO // NQ - 1))
                        osb = s4o.tile([P, 512], bf16, tag="osb")
                        nc.vector.tensor_tensor(osb, ps_o, accs[(t, oc)],
                                                ALU.add)
                        nc.sync.dma_start(
                            out_d.ap()[P * t:P * (t + 1),
                                       512 * oc:512 * (oc + 1)], osb)
            s4_ctx.__exit__(None, None, None)
            _wo_free()

    nc.compile()
    return nc


def shard_inputs(inputs, S=2048, HS=2048):
    """Full problem inputs -> list of 8 per-core in_maps (host-side prep)."""
    x = np.asarray(inputs["x"], np.float32)
    cos = np.asarray(inputs["cos"], np.float32)
    sin = np.asarray(inputs["sin"], np.float32)
    wq = np.asarray(inputs["wq"], np.float32)
    wk = np.asarray(inputs["wk"], np.float32)
    wv = np.asarray(inputs["wv"], np.float32)
    wo = np.asarray(inputs["wo"], np.float32)
    qw = np.asarray(inputs["q_norm_w"], np.float32)
    kw = np.asarray(inputs["k_norm_w"], np.float32)

    T = S // P
    M = 2 * T

    KT = HS // P
    xT_t = np.ascontiguousarray(
        x.reshape(M, P, KT, P).transpose(0, 3, 2, 1).reshape(M, P, HS)).astype(BF16)

    sgn = np.concatenate([-np.ones(64, np.float32), np.ones(64, np.float32)])
    scale = 1.0 / np.sqrt(D)

    def tile_p(a):
        # [(n*P), inner] row-major -> [P, n*inner] partition-major
        n = a.shape[0] // P
        return np.ascontiguousarray(
            a.reshape(n, P, a.shape[1]).transpose(1, 0, 2).reshape(P, -1))

    def fold(w, s):
        w_rot = np.concatenate([w[64:], w[:64]])
        c = tile_p((cos * w[None, :] * s).astype(np.float32)).astype(BF16)
        sn = tile_p((sin * (w_rot * sgn)[None, :] * s).astype(np.float32)).astype(BF16)
        return c, sn

    cosq, sinq = fold(qw, scale)
    cosk, sink = fold(kw, 1.0)

    r = np.arange(P)[:, None]
    t = np.arange(P)[None, :]
    tri = (r <= t).astype(BF16)

    onesq = np.ones((P, P), BF16)
    ident = np.eye(P, dtype=np.float32).astype(BF16)
    woT = tile_p(np.ascontiguousarray(wo.T)).astype(BF16)

    in_maps = []
    for c in range(N_CORES):
        kvh = c // 2
        wq_c = wq[2 * c * D:(2 * c + 2) * D]       # [256, HS]
        # kv-split: even cores project k, odd cores project v; the pair
        # exchanges results on-device.
        wx = (wk if c % 2 == 0 else wv)[kvh * D:(kvh + 1) * D]  # [128, HS]
        wqkv = np.concatenate([wq_c, wx], axis=0)  # [384, HS]
        wqkvT = tile_p(np.ascontiguousarray(wqkv.T)).astype(BF16)  # [P, KT*384]
        in_maps.append({
            "xT": xT_t, "wqkvT": wqkvT, "woT": woT,
            "cosq": cosq, "sinq": sinq, "cosk": cosk, "sink": sink,
            "tri": tri, "onesq": onesq, "ident": ident,
        })
    return in_maps


def assemble(outs, S=2048, HS=2048):
    """Per-core strip outputs -> full [B, S, HS] output."""
    CW = S // 4
    full = np.empty((B, S, HS), np.float32)
    for c in range(N_CORES):
        full[c // 4, (c % 4) * CW:(c % 4 + 1) * CW, :] = \
            np.asarray(outs[c], dtype=np.float32)
    return full


_CACHE = {}


def _get_compiled(S=2048, HS=2048):
    key = (S, HS)
    if key not in _CACHE:
        _CACHE[key] = build(S, HS)
    return _CACHE[key]


def _ensure_ntff_hook():
    """The image's antenv lacks axon_hooks; synthesize it so trace=True works."""
    import types
    try:
        from antenv.axon_hooks import get_axon_ntff_profile_hook  # noqa: F401
        return
    except ImportError:
        pass
    import antenv
    from trn_agent_boot.trn_boot import _ntff_profile_via_ctypes
    mod = types.ModuleType("antenv.axon_hooks")
    mod._hook = _ntff_profile_via_ctypes("/opt/axon/libaxon_pjrt.so")
    mod.set_axon_ntff_profile_hook = lambda h: setattr(mod, "_hook", h)
    mod.get_axon_ntff_profile_hook = lambda: mod._hook
    sys.modules["antenv.axon_hooks"] = mod
    antenv.axon_hooks = mod


def run(inputs, S=2048, HS=2048, trace=False, tmpdir=None):
    import concourse.bass_utils as bu
    if trace:
        _ensure_ntff_hook()
        bu.upload_artifacts = lambda d: ""  # no artifact bucket in this container
    nc = _get_compiled(S, HS)
    in_maps = shard_inputs(inputs, S, HS)
    res = bu.run_bass_kernel_spmd(nc, in_maps, core_ids=list(range(N_CORES)),
                                  trace=trace, tmpdir=tmpdir)
    out = assemble([r["out"] for r in res.results], S, HS)
    return out, res.exec_time_ns


def kernel(**inputs):
    out, _ = run(inputs)
    return out


# revision 29
# speedup vs baseline: 1.1551x; 1.1551x over previous
"""Trainium2 Bass kernel for GQA attention block (B=2, S=2048, HS=2048, H=16, HKV=4, D=128).

Strategy (8 NeuronCores, SPMD):
  - Head-parallel: core c computes q-heads {2c, 2c+1} and kv-head c//2 for BOTH batches.
  - Fused QKV projection: one 512-wide rhs stream [q0|q1|k|v] per contraction tile.
  - Per-head RMS norm + RoPE in [tok, d] layout; rope reads PSUM directly on VectorE;
    the norm multiply is folded into the PE transpose by using diag(1/rms) as the
    transpose's streaming operand (norm weights and 1/sqrt(D) folded into
    host-precomputed cos/sin tables).
  - Causal flash attention in transposed layout: S^T = K_rope @ Q_rope^T ([kv, q]),
    exp on ScalarE (no max subtraction needed: |scores| <= sqrt(D)), diagonal blocks
    narrowed to the causal triangle (matmul/exp/mask only over valid columns),
    O^T = V^T @ P^T accumulated in PSUM. Softmax denominators: GpSimd C-axis
    tensor_reduce for full blocks (gathered in a [16,512] tile, folded in with one
    small matmul), narrowed ones-matmuls for diagonal blocks.
  - One 8-rank AllToAll per head redistributes head-shards -> (batch, seq-strip)
    shards; output projection per strip; host concatenates the 8 strips.
"""

import sys

sys.path.insert(0, "/opt/trn_rl_repo")

import numpy as np
import ml_dtypes

BF16 = ml_dtypes.bfloat16

B, H, HKV, D = 2, 16, 4, 128
EPS = 1e-6
P = 128
N_CORES = 8


def build(S=2048, HS=2048):
    """Build + compile the SPMD graph. Returns the Bacc module."""
    import concourse.bacc as bacc
    import concourse.tile as tile
    import concourse.mybir as mybir

    dt = mybir.dt
    f32 = dt.float32
    bf16 = dt.bfloat16
    AF = mybir.ActivationFunctionType
    ALU = mybir.AluOpType
    AX = mybir.AxisListType

    T = S // P          # tok tiles per batch
    M = 2 * T           # tok tiles total (2 batches)
    KT = HS // P        # contraction tiles for qkv projection
    KO = (H * D) // P   # contraction tiles for o projection (16)
    CW = S // 4         # q-chunk width == strip width
    CB = CW // P        # kv blocks per chunk step
    OCH = HS // 512     # output column chunks
    NQ = 2              # q heads per core

    nc = bacc.Bacc("TRN2", target_bir_lowering=False, debug=False,
                   enable_asserts=True, num_devices=N_CORES)

    xT = nc.dram_tensor("xT", [M, P, HS], bf16, kind="ExternalInput")
    wqkvT = nc.dram_tensor("wqkvT", [P, KT * 512], bf16, kind="ExternalInput")
    woT = nc.dram_tensor("woT", [P, KO * HS], bf16, kind="ExternalInput")
    cosq_d = nc.dram_tensor("cosq", [P, T * D], bf16, kind="ExternalInput")
    sinq_d = nc.dram_tensor("sinq", [P, T * D], bf16, kind="ExternalInput")
    cosk_d = nc.dram_tensor("cosk", [P, T * D], bf16, kind="ExternalInput")
    sink_d = nc.dram_tensor("sink", [P, T * D], bf16, kind="ExternalInput")
    tri_d = nc.dram_tensor("tri", [P, P], bf16, kind="ExternalInput")
    onesq_d = nc.dram_tensor("onesq", [P, P], bf16, kind="ExternalInput")
    ident_d = nc.dram_tensor("ident", [P, P], bf16, kind="ExternalInput")
    out_d = nc.dram_tensor("out", [CW, HS], bf16, kind="ExternalOutput")

    with tile.TileContext(nc) as tc:
        with tc.tile_pool(name="const", bufs=1) as cpool, \
             tc.tile_pool(name="weights", bufs=1) as wpool, \
             tc.tile_pool(name="qkv", bufs=1) as qkvpool, \
             tc.tile_pool(name="dram", bufs=1, space="DRAM") as dpool:

            # startup-critical DMA order: wqkv in per-k pieces so the first
            # projection matmuls can start as soon as k=0 lands; rope tables and
            # ident before the first tile's norm/rope; attention-only constants
            # (tri/onesq) last.
            wqkv_sb = wpool.tile([P, KT, 512], bf16, name="wqkv_sb")
            wqkv_src = wqkvT.ap().rearrange("p (k f) -> p k f", k=KT)

            cosq_sb = cpool.tile([P, T, D], bf16, name="cosq_sb")
            sinq_sb = cpool.tile([P, T, D], bf16, name="sinq_sb")
            cosk_sb = cpool.tile([P, T, D], bf16, name="cosk_sb")
            sink_sb = cpool.tile([P, T, D], bf16, name="sink_sb")
            tri_sb = cpool.tile([P, P], bf16, name="tri_sb")
            onesq_sb = cpool.tile([P, P], bf16, name="onesq_sb")
            ident_sb = cpool.tile([P, P], bf16, name="ident_sb")
            eps_sb = cpool.tile([P, 1], f32, name="eps_sb")
            nc.gpsimd.memset(eps_sb[:], EPS)
            scr_sb = cpool.tile([P, 1], f32, name="scr_sb")
            # prewarm the ACT Exp table so its load isn't on the critical path
            # at the stage-A -> attention boundary
            nc.scalar.activation(scr_sb, eps_sb, AF.Exp)

            # per-batch tiles so batch-0 attention can overlap batch-1 stage A
            qT_b = [qkvpool.tile([P, NQ, S], bf16, name=f"qT_b{i}")
                    for i in range(2)]
            kT_b = [qkvpool.tile([P, S], bf16, name=f"kT_b{i}")
                    for i in range(2)]
            v_b = [qkvpool.tile([P, T, D], bf16, name=f"v_b{i}")
                   for i in range(2)]

            a2a_in = [dpool.tile([1024, CW], bf16, name=f"a2a_in{h}")
                      for h in range(NQ)]
            a2a_out = [dpool.tile([1024, CW], bf16, name=f"a2a_out{h}")
                       for h in range(NQ)]

            # ---------------- stage 1+2: QKV projection, RMS norm, RoPE, transpose
            with tc.tile_pool(name="s12", bufs=2) as s12, \
                 tc.tile_pool(name="xin", bufs=6) as xin, \
                 tc.tile_pool(name="ps12", bufs=2, space="PSUM") as ps12:
                xms = {}

                def load_xm(m):
                    t_ = xin.tile([P, KT, P], bf16, tag="xm", name=f"xm{m}")
                    nc.sync.dma_start(t_[:],
                                      xT.ap()[m].rearrange("p (k t) -> p k t", k=KT))
                    xms[m] = t_

                load_xm(0)
                for k in range(KT):
                    nc.sync.dma_start(wqkv_sb[:, k, :], wqkv_src[:, k, :])
                nc.sync.dma_start(cosq_sb[:],
                                  cosq_d.ap().rearrange("p (t d) -> p t d", t=T))
                nc.sync.dma_start(sinq_sb[:],
                                  sinq_d.ap().rearrange("p (t d) -> p t d", t=T))
                nc.sync.dma_start(cosk_sb[:],
                                  cosk_d.ap().rearrange("p (t d) -> p t d", t=T))
                nc.sync.dma_start(sink_sb[:],
                                  sink_d.ap().rearrange("p (t d) -> p t d", t=T))
                nc.sync.dma_start(ident_sb[:], ident_d.ap())
                for m in range(1, 5):
                    load_xm(m)
                nc.sync.dma_start(tri_sb[:], tri_d.ap())
                nc.sync.dma_start(onesq_sb[:], onesq_d.ap())

                pend = None  # previous tile's (ro, rok, diagr, b, col): transposes
                # are deferred one iteration so the norm/rope chain of tile m
                # overlaps the projection matmuls of tile m+1 (keeps PE dense).

                def emit_transposes(p):
                    ro_, rok_, diagr_, b_, col_ = p
                    for idx in range(3):
                        srct = [ro_[:, 0:P], ro_[:, P:2 * P], rok_][idx]
                        dst = qT_b[b_][:, idx, col_:col_ + P] if idx < 2 \
                            else kT_b[b_][:, col_:col_ + P]
                        tp = ps12.tile([P, P], f32, tag="tp")
                        nc.tensor.matmul(tp, srct, diagr_[:, idx, :],
                                         start=True, stop=True)
                        nc.scalar.copy(dst, tp)

                for m in range(M):
                    b, mm = m // T, m % T
                    if m + 5 < M:
                        load_xm(m + 5)
                    xm = xms.pop(m)
                    ps_qkv = ps12.tile([P, 512], f32, tag="qkv")
                    for k in range(KT):
                        nc.tensor.matmul(ps_qkv, xm[:, k, :], wqkv_sb[:, k, :],
                                         start=(k == 0), stop=(k == KT - 1))
                    # V: plain copy to [tok, d] layout (VectorE; reads PSUM)
                    nc.vector.tensor_copy(v_b[b][:, mm, :], ps_qkv[:, 384:512])
                    col = P * mm
                    # squared-sums for q0|q1|k read straight from PSUM (ScalarE)
                    sq = s12.tile([P, P], bf16, tag="sq")
                    ssum = s12.tile([P, 3], f32, tag="ssum")
                    for idx in range(3):
                        nc.scalar.activation(sq, ps_qkv[:, idx * P:(idx + 1) * P],
                                             AF.Square,
                                             accum_out=ssum[:, idx:idx + 1])
                    rms = s12.tile([P, 3], f32, tag="rms")
                    nc.scalar.activation(rms, ssum, AF.Sqrt,
                                         bias=eps_sb[:], scale=1.0 / D)
                    rinv = s12.tile([P, 3], f32, tag="rinv")
                    nc.vector.reciprocal_approx_fast(rinv, rms)
                    # diag(1/rms) per head: folded into the transpose matmuls
                    diagr = s12.tile([P, 3, P], bf16, tag="diagr")
                    for idx in range(3):
                        nc.vector.tensor_scalar_mul(diagr[:, idx, :], ident_sb,
                                                    rinv[:, idx:idx + 1])
                    # RoPE on raw q/k straight from PSUM; q heads batched via
                    # duplicated tables. (rope commutes with the per-token norm
                    # scalar, which is applied by the diag-transpose below.)
                    ro = s12.tile([P, 2 * P], bf16, tag="ro")
                    ro_v = ro.rearrange("p (h d) -> p h d", h=2)
                    ps_q_h = ps_qkv[:, 0:2 * P].rearrange("p (h d) -> p h d", h=2)
                    cos_b = cosq_sb[:, mm, None, :].to_broadcast((P, 2, D))
                    nc.vector.tensor_tensor(ro_v, ps_q_h, cos_b, ALU.mult)
                    rh = s12.tile([P, 2 * P], bf16, tag="rh")
                    rh_v = rh.rearrange("p (h x d) -> p h x d", h=2, x=2)
                    ps_q_x = ps_qkv[:, 0:2 * P].rearrange("p (h x d) -> p h x d",
                                                          h=2, x=2)
                    sinq_mm = sinq_sb[:, mm, :].rearrange("p (x d) -> p x d", x=2)
                    sinA = sinq_mm[:, None, 0, :].to_broadcast((P, 2, 64))
                    sinB = sinq_mm[:, None, 1, :].to_broadcast((P, 2, 64))
                    nc.vector.tensor_tensor(rh_v[:, :, 0, :], ps_q_x[:, :, 1, :],
                                            sinA, ALU.mult)
                    nc.vector.tensor_tensor(rh_v[:, :, 1, :], ps_q_x[:, :, 0, :],
                                            sinB, ALU.mult)
                    nc.vector.tensor_tensor(ro, ro, rh, ALU.add)
                    # RoPE for k
                    rok = s12.tile([P, P], bf16, tag="rok")
                    nc.vector.tensor_tensor(rok, ps_qkv[:, 2 * P:3 * P],
                                            cosk_sb[:, mm, :], ALU.mult)
                    rhk = s12.tile([P, P], bf16, tag="rhk")
                    nc.vector.tensor_tensor(rhk[:, 0:64], ps_qkv[:, 2 * P + 64:3 * P],
                                            sink_sb[:, mm, 0:64], ALU.mult)
                    nc.vector.tensor_tensor(rhk[:, 64:128], ps_qkv[:, 2 * P:2 * P + 64],
                                            sink_sb[:, mm, 64:128], ALU.mult)
                    nc.vector.tensor_tensor(rok, rok, rhk, ALU.add)
                    if pend is not None:
                        emit_transposes(pend)
                    pend = (ro, rok, diagr, b, col)
                emit_transposes(pend)

            # prefetch o-projection weights during attention
            wo_sb, _wo_free = tc.tile([P, KO, HS], bf16, name="wo_sb")
            wo_src = woT.ap().rearrange("p (k f) -> p k f", k=KO)
            for k4 in range(0, KO, 4):
                nc.sync.dma_start(wo_sb[:, k4:k4 + 4, :], wo_src[:, k4:k4 + 4, :])

            # ---------------- stage 3: causal attention, head-major so each
            # head's AllToAll overlaps the next head's compute
            s4_ctx = tc.tile_pool(name="s4", bufs=1)
            s4 = s4_ctx.__enter__()
            with tc.tile_pool(name="s3", bufs=6) as s3, \
                 tc.tile_pool(name="s3b", bufs=2) as s3b, \
                 tc.tile_pool(name="ps3", bufs=1, space="PSUM") as ps3:
                attn_sb = []
                for h in range(NQ):
                    for bb in range(2):
                        for c in (3, 2, 1, 0):
                            qv = qT_b[bb][:, h, CW * c: CW * (c + 1)]
                            nb = (c + 1) * CB
                            nd = nb - CB  # non-diagonal blocks (full width)
                            o_ps = ps3.tile([P, CW], f32, tag="o", bufs=2)
                            sum_ps = ps3.tile([P, CW], f32, tag="sum", bufs=2)

                            def emit_acc(kb, pT, off, vb=v_b[bb], nb=nb):
                                # denominator + O accumulation for block kb;
                                # deferred one block so the next score matmul
                                # hides the exp/mask latency (keeps PE dense).
                                nc.tensor.matmul(sum_ps[:, off:], onesq_sb,
                                                 pT[:, off:], start=(kb == 0),
                                                 stop=(kb == nb - 1))
                                nc.tensor.matmul(o_ps[:, off:],
                                                 vb[:, kb, :], pT[:, off:],
                                                 start=(kb == 0),
                                                 stop=(kb == nb - 1))

                            pend = None
                            for kb in range(nb):
                                j = kb - nd  # diagonal index, >= 0 for diag blocks
                                off = P * j if j >= 0 else 0
                                s_ps = ps3.tile([P, CW], f32, tag="s", bufs=3)
                                nc.tensor.matmul(
                                    s_ps[:, off:],
                                    kT_b[bb][:, P * kb: P * (kb + 1)],
                                    qv[:, off:], start=True, stop=True)
                                pT = s3.tile([P, CW], bf16, tag="pT")
                                nc.scalar.activation(pT[:, off:], s_ps[:, off:],
                                                     AF.Exp)
                                if j >= 0:
                                    # causal triangle on the first P columns of
                                    # this diagonal block
                                    nc.vector.tensor_tensor(
                                        pT[:, off:off + P], pT[:, off:off + P],
                                        tri_sb, ALU.mult)
                                if pend is not None:
                                    emit_acc(*pend)
                                pend = (kb, pT, off)
                            emit_acc(*pend)
                            rec = s3b.tile([P, CW], f32, tag="rec")
                            nc.vector.reciprocal_approx_fast(rec, sum_ps)
                            o_sb = s3b.tile([P, CW], bf16, tag="o_sb")
                            nc.vector.tensor_tensor(o_sb, o_ps, rec, ALU.mult)
                            r0 = P * (4 * bb + c)
                            nc.sync.dma_start(a2a_in[h][r0:r0 + P, :], o_sb)
                    nc.gpsimd.collective_compute(
                        "AllToAll", mybir.AluOpType.bypass,
                        ins=[a2a_in[h][:].opt()],
                        outs=[a2a_out[h][:].opt()],
                        replica_groups=[list(range(N_CORES))],
                    )
                    a_sb = s4.tile([P, KO // NQ, CW], bf16, name=f"attn_sb{h}")
                    a2a_view = a2a_out[h][:].rearrange("(k p) t -> p k t", p=P)
                    for k8 in range(KO // NQ):
                        nc.sync.dma_start(a_sb[:, k8, :], a2a_view[:, k8, :])
                    attn_sb.append(a_sb)

            # ---------------- stage 4: output projection for this core's strip
            with tc.tile_pool(name="s4o", bufs=2) as s4o, \
                 tc.tile_pool(name="ps4", bufs=2, space="PSUM") as ps4:
                accs = {}
                for t in range(CW // P):
                    for oc in range(OCH):
                        ps_o = ps4.tile([P, 512], f32, tag="oproj")
                        for k8 in range(KO // NQ):
                            nc.tensor.matmul(
                                ps_o, attn_sb[0][:, k8, P * t:P * (t + 1)],
                                wo_sb[:, NQ * k8, 512 * oc:512 * (oc + 1)],
                                start=(k8 == 0), stop=(k8 == KO // NQ - 1))
                        acc = s4o.tile([P, 512], bf16, tag="acc", bufs=16)
                        nc.scalar.copy(acc, ps_o)
                        accs[(t, oc)] = acc
                for t in range(CW // P):
                    for oc in range(OCH):
                        ps_o = ps4.tile([P, 512], f32, tag="oproj")
                        for k8 in range(KO // NQ):
                            nc.tensor.matmul(
                                ps_o, attn_sb[1][:, k8, P * t:P * (t + 1)],
                                wo_sb[:, NQ * k8 + 1, 512 * oc:512 * (oc + 1)],
                                start=(k8 == 0), stop=(k8 == KO // NQ - 1))
                        osb = s4o.tile([P, 512], bf16, tag="osb")
                        nc.vector.tensor_tensor(osb, ps_o, accs[(t, oc)],
                                                ALU.add)
                        nc.sync.dma_start(
                            out_d.ap()[P * t:P * (t + 1),
                                       512 * oc:512 * (oc + 1)], osb)
            s4_ctx.__exit__(None, None, None)
            _wo_free()

    nc.compile()
    return nc


def shard_inputs(inputs, S=2048, HS=2048):
    """Full problem inputs -> list of 8 per-core in_maps (host-side prep)."""
    x = np.asarray(inputs["x"], np.float32)
    cos = np.asarray(inputs["cos"], np.float32)
    sin = np.asarray(inputs["sin"], np.float32)
    wq = np.asarray(inputs["wq"], np.float32)
    wk = np.asarray(inputs["wk"], np.float32)
    wv = np.asarray(inputs["wv"], np.float32)
    wo = np.asarray(inputs["wo"], np.float32)
    qw = np.asarray(inputs["q_norm_w"], np.float32)
    kw = np.asarray(inputs["k_norm_w"], np.float32)

    T = S // P
    M = 2 * T

    KT = HS // P
    xT_t = np.ascontiguousarray(
        x.reshape(M, P, KT, P).transpose(0, 3, 2, 1).reshape(M, P, HS)).astype(BF16)

    sgn = np.concatenate([-np.ones(64, np.float32), np.ones(64, np.float32)])
    scale = 1.0 / np.sqrt(D)

    def tile_p(a):
        # [(n*P), inner] row-major -> [P, n*inner] partition-major
        n = a.shape[0] // P
        return np.ascontiguousarray(
            a.reshape(n, P, a.shape[1]).transpose(1, 0, 2).reshape(P, -1))

    def fold(w, s):
        w_rot = np.concatenate([w[64:], w[:64]])
        c = tile_p((cos * w[None, :] * s).astype(np.float32)).astype(BF16)
        sn = tile_p((sin * (w_rot * sgn)[None, :] * s).astype(np.float32)).astype(BF16)
        return c, sn

    cosq, sinq = fold(qw, scale)
    cosk, sink = fold(kw, 1.0)

    r = np.arange(P)[:, None]
    t = np.arange(P)[None, :]
    tri = (r <= t).astype(BF16)

    onesq = np.ones((P, P), BF16)
    ident = np.eye(P, dtype=np.float32).astype(BF16)
    woT = tile_p(np.ascontiguousarray(wo.T)).astype(BF16)

    in_maps = []
    for c in range(N_CORES):
        kvh = c // 2
        wq_c = wq[2 * c * D:(2 * c + 2) * D]       # [256, HS]
        wk_c = wk[kvh * D:(kvh + 1) * D]           # [128, HS]
        wv_c = wv[kvh * D:(kvh + 1) * D]           # [128, HS]
        wqkv = np.concatenate([wq_c, wk_c, wv_c], axis=0)  # [512, HS]
        wqkvT = tile_p(np.ascontiguousarray(wqkv.T)).astype(BF16)  # [P, KT*512]
        in_maps.append({
            "xT": xT_t, "wqkvT": wqkvT, "woT": woT,
            "cosq": cosq, "sinq": sinq, "cosk": cosk, "sink": sink,
            "tri": tri, "onesq": onesq, "ident": ident,
        })
    return in_maps


def assemble(outs, S=2048, HS=2048):
    """Per-core strip outputs -> full [B, S, HS] output."""
    CW = S // 4
    full = np.empty((B, S, HS), np.float32)
    for c in range(N_CORES):
        full[c // 4, (c % 4) * CW:(c % 4 + 1) * CW, :] = \
            np.asarray(outs[c], dtype=np.float32)
    return full


_CACHE = {}


def _get_compiled(S=2048, HS=2048):
    key = (S, HS)
    if key not in _CACHE:
        _CACHE[key] = build(S, HS)
    return _CACHE[key]


def _ensure_ntff_hook():
    """The image's antenv lacks axon_hooks; synthesize it so trace=True works."""
    import types
    try:
        from antenv.axon_hooks import get_axon_ntff_profile_hook  # noqa: F401
        return
    except ImportError:
        pass
    import antenv
    from trn_agent_boot.trn_boot import _ntff_profile_via_ctypes
    mod = types.ModuleType("antenv.axon_hooks")
    mod._hook = _ntff_profile_via_ctypes("/opt/axon/libaxon_pjrt.so")
    mod.set_axon_ntff_profile_hook = lambda h: setattr(mod, "_hook", h)
    mod.get_axon_ntff_profile_hook = lambda: mod._hook
    sys.modules["antenv.axon_hooks"] = mod
    antenv.axon_hooks = mod


def run(inputs, S=2048, HS=2048, trace=False, tmpdir=None):
    import concourse.bass_utils as bu
    if trace:
        _ensure_ntff_hook()
        bu.upload_artifacts = lambda d: ""  # no artifact bucket in this container
    nc = _get_compiled(S, HS)
    in_maps = shard_inputs(inputs, S, HS)
    res = bu.run_bass_kernel_spmd(nc, in_maps, core_ids=list(range(N_CORES)),
                                  trace=trace, tmpdir=tmpdir)
    out = assemble([r["out"] for r in res.results], S, HS)
    return out, res.exec_time_ns


def kernel(**inputs):
    out, _ = run(inputs)
    return out


# revision 31
# speedup vs baseline: 1.1569x; 1.0016x over previous
"""Trainium2 Bass kernel for GQA attention block (B=2, S=2048, HS=2048, H=16, HKV=4, D=128).

Strategy (8 NeuronCores, SPMD):
  - Head-parallel: core c computes q-heads {2c, 2c+1} and kv-head c//2 for BOTH batches.
  - Fused QKV projection: one 512-wide rhs stream [q0|q1|k|v] per contraction tile.
  - Per-head RMS norm + RoPE in [tok, d] layout; rope reads PSUM directly on VectorE;
    the norm multiply is folded into the PE transpose by using diag(1/rms) as the
    transpose's streaming operand (norm weights and 1/sqrt(D) folded into
    host-precomputed cos/sin tables).
  - Causal flash attention in transposed layout: S^T = K_rope @ Q_rope^T ([kv, q]),
    exp on ScalarE (no max subtraction needed: |scores| <= sqrt(D)), diagonal blocks
    narrowed to the causal triangle (matmul/exp/mask only over valid columns),
    O^T = V^T @ P^T accumulated in PSUM. Softmax denominators: GpSimd C-axis
    tensor_reduce for full blocks (gathered in a [16,512] tile, folded in with one
    small matmul), narrowed ones-matmuls for diagonal blocks.
  - One 8-rank AllToAll per head redistributes head-shards -> (batch, seq-strip)
    shards; output projection per strip; host concatenates the 8 strips.
"""

import sys

sys.path.insert(0, "/opt/trn_rl_repo")

import numpy as np
import ml_dtypes

BF16 = ml_dtypes.bfloat16

B, H, HKV, D = 2, 16, 4, 128
EPS = 1e-6
P = 128
N_CORES = 8


def build(S=2048, HS=2048):
    """Build + compile the SPMD graph. Returns the Bacc module."""
    import concourse.bacc as bacc
    import concourse.tile as tile
    import concourse.mybir as mybir

    dt = mybir.dt
    f32 = dt.float32
    bf16 = dt.bfloat16
    AF = mybir.ActivationFunctionType
    ALU = mybir.AluOpType
    AX = mybir.AxisListType

    T = S // P          # tok tiles per batch
    M = 2 * T           # tok tiles total (2 batches)
    KT = HS // P        # contraction tiles for qkv projection
    KO = (H * D) // P   # contraction tiles for o projection (16)
    CW = S // 4         # q-chunk width == strip width
    CB = CW // P        # kv blocks per chunk step
    OCH = HS // 512     # output column chunks
    NQ = 2              # q heads per core

    nc = bacc.Bacc("TRN2", target_bir_lowering=False, debug=False,
                   enable_asserts=True, num_devices=N_CORES)

    xT = nc.dram_tensor("xT", [M, P, HS], bf16, kind="ExternalInput")
    wqkvT = nc.dram_tensor("wqkvT", [P, KT * 512], bf16, kind="ExternalInput")
    woT = nc.dram_tensor("woT", [P, KO * HS], bf16, kind="ExternalInput")
    cosq_d = nc.dram_tensor("cosq", [P, T * D], bf16, kind="ExternalInput")
    sinq_d = nc.dram_tensor("sinq", [P, T * D], bf16, kind="ExternalInput")
    cosk_d = nc.dram_tensor("cosk", [P, T * D], bf16, kind="ExternalInput")
    sink_d = nc.dram_tensor("sink", [P, T * D], bf16, kind="ExternalInput")
    tri_d = nc.dram_tensor("tri", [P, P], bf16, kind="ExternalInput")
    onesq_d = nc.dram_tensor("onesq", [P, P], bf16, kind="ExternalInput")
    ident_d = nc.dram_tensor("ident", [P, P], bf16, kind="ExternalInput")
    out_d = nc.dram_tensor("out", [CW, HS], bf16, kind="ExternalOutput")

    with tile.TileContext(nc) as tc:
        with tc.tile_pool(name="const", bufs=1) as cpool, \
             tc.tile_pool(name="weights", bufs=1) as wpool, \
             tc.tile_pool(name="qkv", bufs=1) as qkvpool, \
             tc.tile_pool(name="dram", bufs=1, space="DRAM") as dpool:

            # startup-critical DMA order: wqkv in per-k pieces so the first
            # projection matmuls can start as soon as k=0 lands; rope tables and
            # ident before the first tile's norm/rope; attention-only constants
            # (tri/onesq) last.
            wqkv_sb = wpool.tile([P, KT, 512], bf16, name="wqkv_sb")
            wqkv_src = wqkvT.ap().rearrange("p (k f) -> p k f", k=KT)

            cosq_sb = cpool.tile([P, T, D], bf16, name="cosq_sb")
            sinq_sb = cpool.tile([P, T, D], bf16, name="sinq_sb")
            cosk_sb = cpool.tile([P, T, D], bf16, name="cosk_sb")
            sink_sb = cpool.tile([P, T, D], bf16, name="sink_sb")
            tri_sb = cpool.tile([P, P], bf16, name="tri_sb")
            onesq_sb = cpool.tile([P, P], bf16, name="onesq_sb")
            ident_sb = cpool.tile([P, P], bf16, name="ident_sb")
            eps_sb = cpool.tile([P, 1], f32, name="eps_sb")
            nc.gpsimd.memset(eps_sb[:], EPS)
            scr_sb = cpool.tile([P, 1], f32, name="scr_sb")
            # prewarm the ACT Exp table so its load isn't on the critical path
            # at the stage-A -> attention boundary
            nc.scalar.activation(scr_sb, eps_sb, AF.Exp)

            # per-batch tiles so batch-0 attention can overlap batch-1 stage A
            qT_b = [qkvpool.tile([P, NQ, S], bf16, name=f"qT_b{i}")
                    for i in range(2)]
            kT_b = [qkvpool.tile([P, S], bf16, name=f"kT_b{i}")
                    for i in range(2)]
            v_b = [qkvpool.tile([P, T, D], bf16, name=f"v_b{i}")
                   for i in range(2)]

            a2a_in = [dpool.tile([1024, CW], bf16, name=f"a2a_in{h}")
                      for h in range(NQ)]
            a2a_out = [dpool.tile([1024, CW], bf16, name=f"a2a_out{h}")
                       for h in range(NQ)]

            # ---------------- stage 1+2: QKV projection, RMS norm, RoPE, transpose
            with tc.tile_pool(name="s12", bufs=2) as s12, \
                 tc.tile_pool(name="xin", bufs=6) as xin, \
                 tc.tile_pool(name="ps12", bufs=2, space="PSUM") as ps12:
                xms = {}

                def load_xm(m):
                    t_ = xin.tile([P, KT, P], bf16, tag="xm", name=f"xm{m}")
                    nc.sync.dma_start(t_[:],
                                      xT.ap()[m].rearrange("p (k t) -> p k t", k=KT))
                    xms[m] = t_

                load_xm(0)
                for k in range(KT):
                    nc.sync.dma_start(wqkv_sb[:, k, :], wqkv_src[:, k, :])
                nc.sync.dma_start(cosq_sb[:],
                                  cosq_d.ap().rearrange("p (t d) -> p t d", t=T))
                nc.sync.dma_start(sinq_sb[:],
                                  sinq_d.ap().rearrange("p (t d) -> p t d", t=T))
                nc.sync.dma_start(cosk_sb[:],
                                  cosk_d.ap().rearrange("p (t d) -> p t d", t=T))
                nc.sync.dma_start(sink_sb[:],
                                  sink_d.ap().rearrange("p (t d) -> p t d", t=T))
                nc.sync.dma_start(ident_sb[:], ident_d.ap())
                for m in range(1, 5):
                    load_xm(m)
                nc.sync.dma_start(tri_sb[:], tri_d.ap())
                nc.sync.dma_start(onesq_sb[:], onesq_d.ap())

                pend = None  # previous tile's (ro, rok, diagr, b, col): transposes
                # are deferred one iteration so the norm/rope chain of tile m
                # overlaps the projection matmuls of tile m+1 (keeps PE dense).

                def emit_transposes(p):
                    ro_, rok_, diagr_, b_, col_ = p
                    for idx in range(3):
                        srct = [ro_[:, 0:P], ro_[:, P:2 * P], rok_][idx]
                        dst = qT_b[b_][:, idx, col_:col_ + P] if idx < 2 \
                            else kT_b[b_][:, col_:col_ + P]
                        tp = ps12.tile([P, P], f32, tag="tp")
                        nc.tensor.matmul(tp, srct, diagr_[:, idx, :],
                                         start=True, stop=True)
                        nc.scalar.copy(dst, tp)

                for m in range(M):
                    b, mm = m // T, m % T
                    if m + 5 < M:
                        load_xm(m + 5)
                    xm = xms.pop(m)
                    ps_qkv = ps12.tile([P, 512], f32, tag="qkv")
                    for k in range(KT):
                        nc.tensor.matmul(ps_qkv, xm[:, k, :], wqkv_sb[:, k, :],
                                         start=(k == 0), stop=(k == KT - 1))
                    # V: plain copy to [tok, d] layout (VectorE; reads PSUM)
                    nc.vector.tensor_copy(v_b[b][:, mm, :], ps_qkv[:, 384:512])
                    col = P * mm
                    # squared-sums for q0|q1|k read straight from PSUM (ScalarE)
                    sq = s12.tile([P, P], bf16, tag="sq")
                    ssum = s12.tile([P, 3], f32, tag="ssum")
                    for idx in range(3):
                        nc.scalar.activation(sq, ps_qkv[:, idx * P:(idx + 1) * P],
                                             AF.Square,
                                             accum_out=ssum[:, idx:idx + 1])
                    rms = s12.tile([P, 3], f32, tag="rms")
                    nc.scalar.activation(rms, ssum, AF.Sqrt,
                                         bias=eps_sb[:], scale=1.0 / D)
                    rinv = s12.tile([P, 3], f32, tag="rinv")
                    nc.vector.reciprocal_approx_fast(rinv, rms)
                    # diag(1/rms) per head: folded into the transpose matmuls
                    diagr = s12.tile([P, 3, P], bf16, tag="diagr")
                    for idx in range(3):
                        nc.vector.tensor_scalar_mul(diagr[:, idx, :], ident_sb,
                                                    rinv[:, idx:idx + 1])
                    # RoPE on raw q/k straight from PSUM; q heads batched via
                    # duplicated tables. (rope commutes with the per-token norm
                    # scalar, which is applied by the diag-transpose below.)
                    ro = s12.tile([P, 2 * P], bf16, tag="ro")
                    ro_v = ro.rearrange("p (h d) -> p h d", h=2)
                    ps_q_h = ps_qkv[:, 0:2 * P].rearrange("p (h d) -> p h d", h=2)
                    cos_b = cosq_sb[:, mm, None, :].to_broadcast((P, 2, D))
                    nc.vector.tensor_tensor(ro_v, ps_q_h, cos_b, ALU.mult)
                    rh = s12.tile([P, 2 * P], bf16, tag="rh")
                    rh_v = rh.rearrange("p (h x d) -> p h x d", h=2, x=2)
                    ps_q_x = ps_qkv[:, 0:2 * P].rearrange("p (h x d) -> p h x d",
                                                          h=2, x=2)
                    sinq_mm = sinq_sb[:, mm, :].rearrange("p (x d) -> p x d", x=2)
                    sinA = sinq_mm[:, None, 0, :].to_broadcast((P, 2, 64))
                    sinB = sinq_mm[:, None, 1, :].to_broadcast((P, 2, 64))
                    nc.vector.tensor_tensor(rh_v[:, :, 0, :], ps_q_x[:, :, 1, :],
                                            sinA, ALU.mult)
                    nc.vector.tensor_tensor(rh_v[:, :, 1, :], ps_q_x[:, :, 0, :],
                                            sinB, ALU.mult)
                    nc.vector.tensor_tensor(ro, ro, rh, ALU.add)
                    # RoPE for k
                    rok = s12.tile([P, P], bf16, tag="rok")
                    nc.vector.tensor_tensor(rok, ps_qkv[:, 2 * P:3 * P],
                                            cosk_sb[:, mm, :], ALU.mult)
                    rhk = s12.tile([P, P], bf16, tag="rhk")
                    nc.vector.tensor_tensor(rhk[:, 0:64], ps_qkv[:, 2 * P + 64:3 * P],
                                            sink_sb[:, mm, 0:64], ALU.mult)
                    nc.vector.tensor_tensor(rhk[:, 64:128], ps_qkv[:, 2 * P:2 * P + 64],
                                            sink_sb[:, mm, 64:128], ALU.mult)
                    nc.vector.tensor_tensor(rok, rok, rhk, ALU.add)
                    if pend is not None:
                        emit_transposes(pend)
                    pend = (ro, rok, diagr, b, col)
                emit_transposes(pend)

            # prefetch o-projection weights during attention
            wo_sb, _wo_free = tc.tile([P, KO, HS], bf16, name="wo_sb")
            wo_src = woT.ap().rearrange("p (k f) -> p k f", k=KO)
            for k4 in range(0, KO, 4):
                nc.sync.dma_start(wo_sb[:, k4:k4 + 4, :], wo_src[:, k4:k4 + 4, :])

            # ---------------- stage 3: causal attention, head-major so each
            # head's AllToAll overlaps the next head's compute
            s4_ctx = tc.tile_pool(name="s4", bufs=1)
            s4 = s4_ctx.__enter__()
            with tc.tile_pool(name="s3", bufs=6) as s3, \
                 tc.tile_pool(name="s3b", bufs=2) as s3b, \
                 tc.tile_pool(name="ps3", bufs=1, space="PSUM") as ps3:
                attn_sb = []
                for h in range(NQ):
                    for bb in range(2):
                        for c in (3, 2, 1, 0):
                            qv = qT_b[bb][:, h, CW * c: CW * (c + 1)]
                            nb = (c + 1) * CB
                            nd = nb - CB  # non-diagonal blocks (full width)
                            o_ps = ps3.tile([P, CW], f32, tag="o", bufs=2)
                            sum_ps = ps3.tile([P, CW], f32, tag="sum", bufs=2)

                            def emit_acc(kb, pT, off, vb=v_b[bb], nb=nb):
                                # denominator + O accumulation for block kb;
                                # deferred one block so the next score matmul
                                # hides the exp/mask latency (keeps PE dense).
                                nc.tensor.matmul(sum_ps[:, off:], onesq_sb,
                                                 pT[:, off:], start=(kb == 0),
                                                 stop=(kb == nb - 1))
                                nc.tensor.matmul(o_ps[:, off:],
                                                 vb[:, kb, :], pT[:, off:],
                                                 start=(kb == 0),
                                                 stop=(kb == nb - 1))

                            pend = None
                            for kb in range(nb):
                                j = kb - nd  # diagonal index, >= 0 for diag blocks
                                off = P * j if j >= 0 else 0
                                s_ps = ps3.tile([P, CW], f32, tag="s", bufs=3)
                                nc.tensor.matmul(
                                    s_ps[:, off:],
                                    kT_b[bb][:, P * kb: P * (kb + 1)],
                                    qv[:, off:], start=True, stop=True)
                                pT = s3.tile([P, CW], bf16, tag="pT")
                                nc.scalar.activation(pT[:, off:], s_ps[:, off:],
                                                     AF.Exp)
                                if j >= 0:
                                    # causal triangle on the first P columns of
                                    # this diagonal block
                                    nc.vector.tensor_tensor(
                                        pT[:, off:off + P], pT[:, off:off + P],
                                        tri_sb, ALU.mult)
                                if pend is not None:
                                    emit_acc(*pend)
                                pend = (kb, pT, off)
                            emit_acc(*pend)
                            rec = s3b.tile([P, CW], f32, tag="rec")
                            nc.vector.reciprocal_approx_fast(rec, sum_ps)
                            o_sb = s3b.tile([P, CW], bf16, tag="o_sb")
                            nc.vector.tensor_tensor(o_sb, o_ps, rec, ALU.mult)
                            r0 = P * (4 * bb + c)
                            nc.sync.dma_start(a2a_in[h][r0:r0 + P, :], o_sb)
                    nc.gpsimd.collective_compute(
                        "AllToAll", mybir.AluOpType.bypass,
                        ins=[a2a_in[h][:].opt()],
                        outs=[a2a_out[h][:].opt()],
                        replica_groups=[list(range(N_CORES))],
                    )
                    a_sb = s4.tile([P, KO // NQ, CW], bf16, name=f"attn_sb{h}")
                    a2a_view = a2a_out[h][:].rearrange("(k p) t -> p k t", p=P)
                    for k8 in range(KO // NQ):
                        nc.sync.dma_start(a_sb[:, k8, :], a2a_view[:, k8, :])
                    attn_sb.append(a_sb)

            # ---------------- stage 4: output projection for this core's strip
            with tc.tile_pool(name="s4o", bufs=2) as s4o, \
                 tc.tile_pool(name="ps4", bufs=2, space="PSUM") as ps4:
                accs = {}
                for t in range(CW // P):
                    for oc in range(OCH):
                        ps_o = ps4.tile([P, 512], f32, tag="oproj")
                        for k8 in range(KO // NQ):
                            nc.tensor.matmul(
                                ps_o, attn_sb[0][:, k8, P * t:P * (t + 1)],
                                wo_sb[:, NQ * k8, 512 * oc:512 * (oc + 1)],
                                start=(k8 == 0), stop=(k8 == KO // NQ - 1))
                        acc = s4o.tile([P, 512], bf16, tag="acc", bufs=16)
                        nc.scalar.copy(acc, ps_o)
                        accs[(t, oc)] = acc
                for t in range(CW // P):
                    for oc in range(OCH):
                        ps_o = ps4.tile([P, 512], f32, tag="oproj")
                        for k8 in range(KO // NQ):
                            nc.tensor.matmul(
                                ps_o, attn_sb[1][:, k8, P * t:P * (t + 1)],
                                wo_sb[:, NQ * k8 + 1, 512 * oc:512 * (oc + 1)],
                                start=(k8 == 0), stop=(k8 == KO // NQ - 1))
                        osb = s4o.tile([P, 512], bf16, tag="osb")
                        nc.vector.tensor_tensor(osb, ps_o, accs[(t, oc)],
                                                ALU.add)
                        nc.sync.dma_start(
                            out_d.ap()[P * t:P * (t + 1),
                                       512 * oc:512 * (oc + 1)], osb)
            s4_ctx.__exit__(None, None, None)
            _wo_free()

    nc.compile()
    return nc


def shard_inputs(inputs, S=2048, HS=2048):
    """Full problem inputs -> list of 8 per-core in_maps (host-side prep)."""
    x = np.asarray(inputs["x"], np.float32)
    cos = np.asarray(inputs["cos"], np.float32)
    sin = np.asarray(inputs["sin"], np.float32)
    wq = np.asarray(inputs["wq"], np.float32)
    wk = np.asarray(inputs["wk"], np.float32)
    wv = np.asarray(inputs["wv"], np.float32)
    wo = np.asarray(inputs["wo"], np.float32)
    qw = np.asarray(inputs["q_norm_w"], np.float32)
    kw = np.asarray(inputs["k_norm_w"], np.float32)

    T = S // P
    M = 2 * T

    KT = HS // P
    xT_t = np.ascontiguousarray(
        x.reshape(M, P, KT, P).transpose(0, 3, 2, 1).reshape(M, P, HS)).astype(BF16)

    sgn = np.concatenate([-np.ones(64, np.float32), np.ones(64, np.float32)])
    scale = 1.0 / np.sqrt(D)

    def tile_p(a):
        # [(n*P), inner] row-major -> [P, n*inner] partition-major
        n = a.shape[0] // P
        return np.ascontiguousarray(
            a.reshape(n, P, a.shape[1]).transpose(1, 0, 2).reshape(P, -1))

    def fold(w, s):
        w_rot = np.concatenate([w[64:], w[:64]])
        c = tile_p((cos * w[None, :] * s).astype(np.float32)).astype(BF16)
        sn = tile_p((sin * (w_rot * sgn)[None, :] * s).astype(np.float32)).astype(BF16)
        return c, sn

    cosq, sinq = fold(qw, scale)
    cosk, sink = fold(kw, 1.0)

    r = np.arange(P)[:, None]
    t = np.arange(P)[None, :]
    tri = (r <= t).astype(BF16)

    onesq = np.ones((P, P), BF16)
    ident = np.eye(P, dtype=np.float32).astype(BF16)
    woT = tile_p(np.ascontiguousarray(wo.T)).astype(BF16)

    in_maps = []
    for c in range(N_CORES):
        kvh = c // 2
        wq_c = wq[2 * c * D:(2 * c + 2) * D]       # [256, HS]
        wk_c = wk[kvh * D:(kvh + 1) * D]           # [128, HS]
        wv_c = wv[kvh * D:(kvh + 1) * D]           # [128, HS]
        wqkv = np.concatenate([wq_c, wk_c, wv_c], axis=0)  # [512, HS]
        wqkvT = tile_p(np.ascontiguousarray(wqkv.T)).astype(BF16)  # [P, KT*512]
        in_maps.append({
            "xT": xT_t, "wqkvT": wqkvT, "woT": woT,
            "cosq": cosq, "sinq": sinq, "cosk": cosk, "sink": sink,
            "tri": tri, "onesq": onesq, "ident": ident,
        })
    return in_maps


def assemble(outs, S=2048, HS=2048):
    """Per-core strip outputs -> full [B, S, HS] output."""
    CW = S // 4
    full = np.empty((B, S, HS), np.float32)
    for c in range(N_CORES):
        full[c // 4, (c % 4) * CW:(c % 4 + 1) * CW, :] = \
            np.asarray(outs[c], dtype=np.float32)
    return full


_CACHE = {}


def _get_compiled(S=2048, HS=2048):
    key = (S, HS)
    if key not in _CACHE:
        _CACHE[key] = build(S, HS)
    return _CACHE[key]


def _ensure_ntff_hook():
    """The image's antenv lacks axon_hooks; synthesize it so trace=True works."""
    import types
    try:
        from antenv.axon_hooks import get_axon_ntff_profile_hook  # noqa: F401
        return
    except ImportError:
        pass
    import antenv
    from trn_agent_boot.trn_boot import _ntff_profile_via_ctypes
    mod = types.ModuleType("antenv.axon_hooks")
    mod._hook = _ntff_profile_via_ctypes("/opt/axon/libaxon_pjrt.so")
    mod.set_axon_ntff_profile_hook = lambda h: setattr(mod, "_hook", h)
    mod.get_axon_ntff_profile_hook = lambda: mod._hook
    sys.modules["antenv.axon_hooks"] = mod
    antenv.axon_hooks = mod


def run(inputs, S=2048, HS=2048, trace=False, tmpdir=None):
    import concourse.bass_utils as bu
    if trace:
        _ensure_ntff_hook()
        bu.upload_artifacts = lambda d: ""  # no artifact bucket in this container
    nc = _get_compiled(S, HS)
    in_maps = shard_inputs(inputs, S, HS)
    res = bu.run_bass_kernel_spmd(nc, in_maps, core_ids=list(range(N_CORES)),
                                  trace=trace, tmpdir=tmpdir)
    out = assemble([r["out"] for r in res.results], S, HS)
    return out, res.exec_time_ns


def kernel(**inputs):
    out, _ = run(inputs)
    return out
